# revision 18
# baseline (speedup 1.0000x reference)
"""Trainium2 Bass kernel for nn_Decay (gated decay-memory block).

  gate  = sigmoid(x @ Wg + bg)
  store = (x @ Wv) * gate * scale          scale = sqrt(1 - decay)
  mem   = decay-scan(store)                y_t = store_t + decay * y_{t-1}
  que   = sigmoid(x @ Wq + bq)
  out   = (mem * que * scale) @ Wo

Sharding (8 cores): core c handles batch b = c//2, token half h = c%2
(2048 output tokens each).  The decay scan needs history: each core
computes a 128-token halo before its token range (zero-padded for h=0,
so all cores run the identical program).  decay^128 ~ 1.4e-3 and only
~7% of it survives in the L2 norm => ~1e-4 rel err.  No collectives.

Precision: pv / gate / out-projection matmuls in bf16 (host-side cast,
Wv/Wo pre-scaled); que matmul in fp8e4m3 with perf_mode=DoubleRow (2
k-chunks per instruction) — the sigmoid damps fp8 quantization and the
measured end-to-end rel err stays ~1.4e-2 < 2e-2.  Wq is host-quantized
at x256 (fp8-range) and the sigmoid activation applies scale=1/256; the
fp8 copy of x is produced on-chip by a DVE cast per block.  The scan
and its carry stay fp32.

Layout: [feature (partitions), token (free)]; x transposed host-side
and fully SBUF-resident (5 block tiles, streamed in during phase A0).

Phases (per core):
  A0..A3: m-quarter q of {Wv*scale, Wg, Wq} resident; token blocks of
          [128, 512, 512, 512, 512]; per block: pv + gate chains
          (N=block), sigmoid, store mul, DVE decay scan, then (output
          blocks only) que DoubleRow chain, l0 = mem*que -> spill
          (bf16).  A0's Wv/Wg load in m-tile chunks so the first
          chains start ~10us in; next quarter's weights prefetch at
          block 2.
  C:      all four e-quarters of Wo*scale live in the (dead) Wv/Wg
          buffers; one pass over 8 token blocks of 256, 16 e-tile
          chains each; outT written bf16 (host upcasts).
"""

import contextlib
import sys

sys.path.insert(0, "/opt/trn_rl_repo")

import numpy as np
import ml_dtypes

import concourse.bass as bass
import concourse.tile as tile
from concourse import bacc, mybir
from concourse.bass_utils import run_bass_kernel_spmd

# Problem constants (hardcoded per harness contract)
B, S, E, M = 4, 4096, 2048, 2048
DECAY = 0.95
SCALE = float(np.sqrt(1.0 - DECAY))

N_CORES = 8
HALO = 128            # halo tokens ahead of each core's range
OUT_T = S // 2        # output tokens per core
T = OUT_T + HALO      # computed tokens per core (2176)
BLKS = [128, 512, 512, 512, 512]          # phase-A token blocks
BSTART = [0, 128, 640, 1152, 1664]
TBC = 256             # phase-C token block
NCB = OUT_T // TBC    # 8
P = 128
EC = E // P           # 16 contraction chunks
MT = M // P           # 16 m tiles
MQ = 4                # m-quarters
MT_Q = MT // MQ       # 4 m-tiles per quarter
MQW = MT_Q * P        # 512
EQ = E // MQ          # 512
F32 = mybir.dt.float32
BF16 = mybir.dt.bfloat16
FP8 = mybir.dt.float8e4
SIG = mybir.ActivationFunctionType.Sigmoid
NPBF16 = ml_dtypes.bfloat16
NPFP8 = ml_dtypes.float8_e4m3

QUE_FP8 = True        # que matmul in fp8 DoubleRow (rel err ~1.4e-2)
WQ_SCALE = 256.0      # host-side Wq multiplier before fp8 cast


def build_module(has_bias, que_fp8):
    nc = bacc.Bacc()

    xT_d = nc.dram_tensor("xT", [E, T], BF16, kind="ExternalInput")
    wv_d = nc.dram_tensor("Wvs", [E, M], BF16, kind="ExternalInput")
    wg_d = nc.dram_tensor("Wg", [E // 2 if que_fp8 else E, M], BF16,
                          kind="ExternalInput")
    if que_fp8:
        wg8_d = nc.dram_tensor("Wg8", [E // 2, M], FP8, kind="ExternalInput")
    wq_d = nc.dram_tensor("Wq", [E, M], FP8 if que_fp8 else BF16,
                          kind="ExternalInput")
    wo_d = nc.dram_tensor("Wos", [M, E], BF16, kind="ExternalInput")
    if has_bias:
        bg_d = nc.dram_tensor("bg", [M], F32, kind="ExternalInput")
        bq_d = nc.dram_tensor("bq", [M], F32, kind="ExternalInput")
    outT_d = nc.dram_tensor("outT", [E, OUT_T], BF16, kind="ExternalOutput")
    l0_buf = nc.dram_tensor("l0_buf", [M, OUT_T], BF16)  # internal spill

    with tile.TileContext(nc) as tc, contextlib.ExitStack() as _st:
        _p = lambda **kw: _st.enter_context(tc.tile_pool(**kw))
        xp = _p(name="xp", bufs=1)
        x8p = _p(name="x8p", bufs=2)
        wvp = _p(name="wvp", bufs=2)
        wgp = _p(name="wgp", bufs=2)
        wqp = _p(name="wqp", bufs=1)
        w8p = _p(name="w8p", bufs=2)
        g8p = _p(name="g8p", bufs=2)
        gsp = _p(name="gsp", bufs=2)
        memp = _p(name="memp", bufs=2)
        stp = _p(name="stp", bufs=2)
        gtp = _p(name="gtp", bufs=2)
        qtp = _p(name="qtp", bufs=3)
        l0p = _p(name="l0p", bufs=3)
        ltp = _p(name="ltp", bufs=2)
        otp = _p(name="otp", bufs=2)
        cp = _p(name="cp", bufs=1)
        ps = _p(name="ps", bufs=2, space="PSUM")
        if True:
            # consts: decay broadcast [:, :512]; bg at [:, 512:512+MT]; bq after
            consts = cp.tile([P, 512 + 2 * MT], F32, tag="consts")
            nc.vector.memset(consts[:, 0:512], DECAY)
            if has_bias:
                nc.sync.dma_start(
                    out=consts[:, 512 : 512 + MT],
                    in_=bg_d.rearrange("(c p) -> p c", p=P),
                )
                nc.sync.dma_start(
                    out=consts[:, 512 + MT : 512 + 2 * MT],
                    in_=bq_d.rearrange("(c p) -> p c", p=P),
                )

            xT_r = xT_d.rearrange("(c p) t -> p c t", p=P)
            l0_r = l0_buf.rearrange("(c p) t -> p c t", p=P)
            outT_r = outT_d.rearrange("(c p) t -> p c t", p=P)

            wtiles = {}

            def load_w(kind, q, chunked=False):
                # weight quarter loads; wo quarters reuse the wv/wg buffers
                if kind in ("v", "g"):
                    pool, wd = ((wvp, wv_d), (wgp, wg_d))[kind == "g"]
                    nec = EC // 2 if (kind == "g" and que_fp8) else EC
                    t = pool.tile([P, nec, MQW], BF16, tag="w" + kind,
                                  name=f"w{kind}{q}")
                    if chunked:
                        # ec-chunked so phase A0's chains (which consume ec
                        # in order) start on the first chunk; lines stay 1KB
                        for ch in range(nec // 4):
                            esl = slice(ch * 4 * P, (ch + 1) * 4 * P)
                            nc.scalar.dma_start(
                                out=t[:, ch * 4 : (ch + 1) * 4, :],
                                in_=wd[esl, q * MQW : (q + 1) * MQW].rearrange(
                                    "(c p) m -> p c m", p=P
                                ),
                            )
                    else:
                        nc.scalar.dma_start(
                            out=t,
                            in_=wd[:, q * MQW : (q + 1) * MQW].rearrange(
                                "(c p) m -> p c m", p=P
                            ),
                        )
                elif kind == "g8":
                    t = w8p.tile([P, EC // 2, MQW], FP8, tag="wg8",
                                 name=f"wg8_{q}")
                    nc.gpsimd.dma_start(
                        out=t,
                        in_=wg8_d[:, q * MQW : (q + 1) * MQW].rearrange(
                            "(c p) m -> p c m", p=P
                        ),
                    )
                elif kind == "q":
                    t = wqp.tile([P, EC, MQW], FP8 if que_fp8 else BF16,
                                 tag="wq", name=f"wq{q}")
                    nc.gpsimd.dma_start(
                        out=t,
                        in_=wq_d[:, q * MQW : (q + 1) * MQW].rearrange(
                            "(c p) m -> p c m", p=P
                        ),
                    )
                else:  # wo e-quarter, on sync; 0/1 in wv bufs, 2/3 in the
                    # dead x01/x2 tiles (wg bufs are half-size now)
                    pool, tag = [(wvp, "wv"), (wvp, "wv"),
                                 (xp, "x01"), (xp, "x2")][q]
                    t = pool.tile([P, MT, EQ], BF16, tag=tag, name=f"wo{q}")
                    nc.sync.dma_start(
                        out=t,
                        in_=wo_d[:, q * EQ : (q + 1) * EQ].rearrange(
                            "(c p) e -> p c e", p=P
                        ),
                    )
                wtiles[(kind, q)] = t


            def emit_gate(q, b, mt, w, mtg, msl, wg, wg8, xb, xo, x8):
                gate = gtp.tile([P, 512], BF16, tag="gt", name=f"g{q}_{b}_{mt}")
                if que_fp8:
                    # ec 8..15 in bf16 (its own PSUM group) ...
                    pg = ps.tile([P, 512], F32, tag="pg", name=f"pg{q}_{b}_{mt}")
                    for ec in range(EC // 2, EC):
                        nc.tensor.matmul(
                            pg[:, :w], lhsT=wg[:, ec - EC // 2, msl],
                            rhs=xb[:, ec, xo : xo + w],
                            start=(ec == EC // 2), stop=(ec == EC - 1),
                        )
                    # ... ec 0..7 as a separate fp8 DoubleRow group (mixing
                    # DR and normal matmuls in one PSUM group miscomputes
                    # on HW)
                    pg8 = ps.tile([P, 512], F32, tag="pg8",
                                  name=f"pg8_{q}_{b}_{mt}")
                    for pr in range(EC // 4):
                        nc.tensor.matmul(
                            pg8[:, :w], lhsT=wg8[:, 2 * pr : 2 * pr + 2, msl],
                            rhs=x8[:, 2 * pr : 2 * pr + 2, :w],
                            start=(pr == 0), stop=(pr == EC // 4 - 1),
                            perf_mode=mybir.MatmulPerfMode.DoubleRow,
                        )
                    g8 = g8p.tile([P, 512], BF16, tag="g8", name=f"g8_{q}_{b}_{mt}")
                    nc.scalar.activation(
                        g8[:, :w], pg8[:, :w], mybir.ActivationFunctionType.Copy,
                    )
                    gs = gsp.tile([P, 512], BF16, tag="gs", name=f"gs{q}_{b}_{mt}")
                    nc.vector.tensor_add(gs[:, :w], pg[:, :w], g8[:, :w])
                    nc.scalar.activation(
                        gate[:, :w], gs[:, :w], SIG,
                        bias=consts[:, 512 + mtg : 512 + mtg + 1]
                        if has_bias else 0.0,
                        scale=1.0 / WQ_SCALE,
                    )
                else:
                    pg = ps.tile([P, 512], F32, tag="pg", name=f"pg{q}_{b}_{mt}")
                    for ec in range(EC):
                        nc.tensor.matmul(
                            pg[:, :w], lhsT=wg[:, ec, msl],
                            rhs=xb[:, ec, xo : xo + w],
                            start=(ec == 0), stop=(ec == EC - 1),
                        )
                    nc.scalar.activation(
                        gate[:, :w], pg[:, :w], SIG,
                        bias=consts[:, 512 + mtg : 512 + mtg + 1]
                        if has_bias else 0.0,
                    )
                return gate

            # startup: phase-0 weights chunked (first-needed first); x0+x1
            # merged into one 640-wide DMA (1.25KB lines)
            def _ldchunk(t, wd, ch):
                esl = slice(ch * 4 * P, (ch + 1) * 4 * P)
                nc.scalar.dma_start(
                    out=t[:, ch * 4 : (ch + 1) * 4, :],
                    in_=wd[esl, 0:MQW].rearrange("(c p) m -> p c m", p=P),
                )

            if que_fp8:
                # interleave so wg0's first bf16 chunk lands before the
                # first gate chain instead of after all of wv0
                wv0 = wvp.tile([P, EC, MQW], BF16, tag="wv", name="wv0")
                wg0 = wgp.tile([P, EC // 2, MQW], BF16, tag="wg", name="wg0")
                for t_, wd_, ch_ in [(wv0, wv_d, 0), (wv0, wv_d, 1),
                                     (wg0, wg_d, 0), (wv0, wv_d, 2),
                                     (wv0, wv_d, 3), (wg0, wg_d, 1)]:
                    _ldchunk(t_, wd_, ch_)
                wtiles[("v", 0)] = wv0
                wtiles[("g", 0)] = wg0
            else:
                load_w("v", 0, chunked=True)
                load_w("g", 0, chunked=True)
            if que_fp8:
                load_w("g8", 0)
            load_w("q", 0)
            x01 = xp.tile([P, EC, 640], BF16, tag="x01", name="x01")
            xres = [x01, x01]
            for b in (2, 3, 4):
                xres.append(
                    xp.tile([P, EC, BLKS[b]], BF16, tag=f"x{b}", name=f"x{b}")
                )

            def load_x(b):
                if b <= 1:
                    # ec-chunked: the first pv chain starts on chunk 0
                    for ch in range(4):
                        nc.sync.dma_start(
                            out=x01[:, ch * 4 : (ch + 1) * 4, :],
                            in_=xT_r[:, ch * 4 : (ch + 1) * 4, 0:640],
                        )
                else:
                    nc.sync.dma_start(
                        out=xres[b],
                        in_=xT_r[:, :, BSTART[b] : BSTART[b] + BLKS[b]],
                    )

            load_x(0)

            x8_tiles = {}

            def emit_cast(cq, cb):
                # cast x -> fp8 one block ahead so the DVE work is off the
                # block's critical path (pg8/pq DoubleRow chains need it)
                cw = BLKS[cb]
                cxb = xres[cb]
                cxo = BSTART[cb] if cb <= 1 else 0
                t = x8p.tile([P, EC, 512], FP8, tag="x8", name=f"x8_{cq}_{cb}")
                nc.vector.tensor_copy(t[:, :, :cw], cxb[:, :, cxo : cxo + cw])
                x8_tiles[(cq, cb)] = t

            if que_fp8:
                emit_cast(0, 0)

            # ---- Phases A0..A3: quarter q of m ----
            for q in range(MQ):
                if q > 0:
                    load_w("q", q)  # waits on phase q-1's last pq chain
                wv, wg, wq = wtiles[("v", q)], wtiles[("g", q)], wtiles[("q", q)]
                wg8 = wtiles.get(("g8", q))
                mem_prev = None
                pw = None
                for b in range(5):
                    w = BLKS[b]
                    xb = xres[b]
                    xo = BSTART[b] if b <= 1 else 0  # offset within x01
                    if q == 0 and b < 3:
                        load_x(b + 2)  # defer bulk x DMA out of startup
                    if q < 3 and b == 2:
                        load_w("v", q + 1)
                        load_w("g", q + 1)
                        if que_fp8:
                            load_w("g8", q + 1)
                    if q == 3 and b == 0:
                        load_w("o", 0)  # wv buf freed after A2
                    if q == 3 and b == 2:
                        load_w("o", 2)  # x01 dead after A3 b1
                    if q == 3 and b == 3:
                        load_w("o", 3)  # x2 dead after A3 b2
                    if que_fp8:
                        x8 = x8_tiles.pop((q, b))
                    mem_t = memp.tile(
                        [P, MT_Q, 512], F32, tag="mem", name=f"mem{q}_{b}"
                    )
                    pvs = []
                    for mt in range(MT_Q):
                        pv = ps.tile(
                            [P, 512], F32, tag="pv", bufs=4, name=f"pv{q}_{b}_{mt}"
                        )
                        msl = slice(mt * P, (mt + 1) * P)
                        for ec in range(EC):
                            nc.tensor.matmul(
                                pv[:, :w], lhsT=wv[:, ec, msl],
                                rhs=xb[:, ec, xo : xo + w],
                                start=(ec == 0), stop=(ec == EC - 1),
                            )
                        pvs.append(pv)
                    if q == 3 and b == 4:
                        load_w("o", 1)  # waits on the pv chains just emitted
                    for mt in range(MT_Q):
                        mtg = q * MT_Q + mt
                        msl = slice(mt * P, (mt + 1) * P)
                        gate = emit_gate(q, b, mt, w, mtg, msl, wg, wg8,
                                         xb, xo, x8)
                        store = stp.tile([P, 512], F32, tag="st", name=f"s{q}_{b}_{mt}")
                        nc.vector.tensor_mul(store[:, :w], pvs[mt][:, :w], gate[:, :w])
                        nc.vector.tensor_tensor_scan(
                            mem_t[:, mt, :w], consts[:, 0:w], store[:, :w],
                            initial=0.0 if b == 0 else mem_prev[:, mt, pw - 1 : pw],
                            op0=mybir.AluOpType.mult, op1=mybir.AluOpType.add,
                        )
                    if que_fp8:
                        nxt = (q, b + 1) if b < 4 else (q + 1, 0)
                        if nxt[0] < MQ:
                            emit_cast(*nxt)
                    if b > 0:
                        osl = slice(BSTART[b] - HALO, BSTART[b] - HALO + w)
                        for mt in range(MT_Q):
                            mtg = q * MT_Q + mt
                            msl = slice(mt * P, (mt + 1) * P)
                            pq = ps.tile(
                                [P, 512], F32, tag="pv", bufs=4, name=f"pq{q}_{b}_{mt}"
                            )
                            if que_fp8:
                                for pr in range(EC // 2):
                                    nc.tensor.matmul(
                                        pq, lhsT=wq[:, 2 * pr : 2 * pr + 2, msl],
                                        rhs=x8[:, 2 * pr : 2 * pr + 2, :],
                                        start=(pr == 0), stop=(pr == EC // 2 - 1),
                                        perf_mode=mybir.MatmulPerfMode.DoubleRow,
                                    )
                            else:
                                for ec in range(EC):
                                    nc.tensor.matmul(
                                        pq[:, :w], lhsT=wq[:, ec, msl],
                                        rhs=xb[:, ec, xo : xo + w],
                                        start=(ec == 0), stop=(ec == EC - 1),
                                    )
                            que = qtp.tile(
                                [P, 512], BF16, tag="qt", name=f"u{q}_{b}_{mt}"
                            )
                            nc.scalar.activation(
                                que[:, :w], pq[:, :w], SIG,
                                bias=consts[:, 512 + MT + mtg : 512 + MT + mtg + 1]
                                if has_bias else 0.0,
                                scale=1.0 / WQ_SCALE if que_fp8 else 1.0,
                            )
                            l0 = l0p.tile([P, 512], BF16, tag="l0", name=f"l{q}_{b}_{mt}")
                            nc.vector.tensor_mul(
                                l0[:, :w], mem_t[:, mt, :w], que[:, :w]
                            )
                            nc.gpsimd.dma_start(
                                out=l0_r[:, mtg : mtg + 1, osl],
                                in_=l0[:, :w].unsqueeze(1),
                            )
                    mem_prev = mem_t
                    pw = w

            # ---- Phase C: output projection, all of Wo resident ----
            wos = [wtiles[("o", eq)] for eq in range(4)]
            for b in range(NCB):
                tsl = slice(b * TBC, (b + 1) * TBC)
                lt = ltp.tile([P, MT, TBC], BF16, tag="lt", name=f"lt{b}")
                nc.sync.dma_start(out=lt, in_=l0_r[:, :, tsl])
                for g in range(4):
                    ot = otp.tile([P, 4, TBC], BF16, tag="ot", name=f"ot{b}_{g}")
                    for j in range(4):
                        et = 4 * g + j
                        eq, esl = et // 4, slice((et % 4) * P, (et % 4 + 1) * P)
                        po = ps.tile([P, 512], F32, tag="pg8",
                                     name=f"po{b}_{et}")[:, :TBC]
                        for mc in range(MT):
                            nc.tensor.matmul(
                                po, lhsT=wos[eq][:, mc, esl], rhs=lt[:, mc, :],
                                start=(mc == 0), stop=(mc == MT - 1),
                            )
                        nc.vector.tensor_copy(ot[:, j, :], po)
                    nc.gpsimd.dma_start(
                        out=outT_r[:, 4 * g : 4 * g + 4, tsl], in_=ot
                    )
    nc.compile()
    return nc


_cached = {}


def _get_module(has_bias):
    key = (has_bias, QUE_FP8)
    if key not in _cached:
        _cached[key] = build_module(has_bias, QUE_FP8)
    return _cached[key]


def _prep_inputs(x, Wv, Wg, bg, Wq, bq, Wo, has_bias):
    """Shard + lay out host-side. Returns per-core input dicts."""
    x = np.asarray(x, dtype=np.float32)
    Wvs = (np.asarray(Wv, dtype=np.float32) * SCALE).astype(NPBF16)
    Wos = (np.asarray(Wo, dtype=np.float32) * SCALE).astype(NPBF16)
    Wgf = np.asarray(Wg, dtype=np.float32)
    if QUE_FP8:
        Wg16 = (Wgf[E // 2 :] * WQ_SCALE).astype(NPBF16)
        Wg8 = (Wgf[: E // 2] * WQ_SCALE).astype(NPFP8)
    else:
        Wg16 = Wgf.astype(NPBF16)
    if QUE_FP8:
        Wqq = (np.asarray(Wq, dtype=np.float32) * WQ_SCALE).astype(NPFP8)
    else:
        Wqq = np.asarray(Wq, dtype=np.float32).astype(NPBF16)
    in_maps = []
    for c in range(N_CORES):
        b, h = c // 2, c % 2
        xTc = np.zeros((E, T), dtype=NPBF16)
        start = h * OUT_T - HALO
        src = np.ascontiguousarray(x[b, max(start, 0) : h * OUT_T + OUT_T].T)
        xTc[:, T - src.shape[1] :] = src.astype(NPBF16)
        m = {"xT": xTc, "Wvs": Wvs, "Wg": Wg16, "Wq": Wqq, "Wos": Wos}
        if QUE_FP8:
            m["Wg8"] = Wg8
        if has_bias:
            m["bg"] = np.ascontiguousarray(bg, dtype=np.float32)
            m["bq"] = np.ascontiguousarray(bq, dtype=np.float32)
        in_maps.append(m)
    return in_maps


def run(x, Wv, Wg, bg, Wq, bq, Wo, trace=False):
    bg = np.asarray(bg, dtype=np.float32)
    bq = np.asarray(bq, dtype=np.float32)
    has_bias = bool(np.any(bg)) or bool(np.any(bq))
    nc = _get_module(has_bias)
    in_maps = _prep_inputs(x, Wv, Wg, bg, Wq, bq, Wo, has_bias)
    res = run_bass_kernel_spmd(
        nc, in_maps, core_ids=list(range(N_CORES)), trace=trace
    )
    out = np.empty((B, S, E), dtype=np.float32)
    for c in range(N_CORES):
        b, h = c // 2, c % 2
        out[b, h * OUT_T : (h + 1) * OUT_T] = (
            res.results[c]["outT"].astype(np.float32).T
        )
    return out, res


def kernel(**inputs):
    out, _ = run(**inputs)
    return out
